# revision 1
# baseline (speedup 1.0000x reference)
"""Trainium2 Bass kernel for nn_Matcher (gnn_message_passing).

Math: for each of N=4*256*256 graphs with node indices n0..n4 in [0,129):
  sim[g,c] = oh(n0) @ A  +  sum_{s=1..4} relu(E1[n_s] + sqrt2*E1[n0]) @ C2_s
where E1 = emb @ W1 (row 128 is zero), A and C2_s are small host-precomputed
tables folding the class branch, GCN propagation weights and W2.
(b1 = b2 = 0 in this problem; scale 2 from h_s is folded into C2_s.)

Device kernel (per core, data-parallel over graphs):
  one-hot columns are built on DVE via tensor_scalar is_equal against an iota
  column, from index rows DMA-replicated across 128 partitions; the PE
  consumes them as bf16 moving operands (1 cycle/column).
"""
import numpy as np
import ml_dtypes

N_CORES = 8
B, H, W_DIM = 4, 256, 256
NTOT = B * H * W_DIM            # 262144 graphs
NCORE = NTOT // N_CORES         # 32768
SC = 2048                       # DVE super-chunk (graphs)
CH = 512                        # matmul chunk (graphs)
SQ2 = float(np.sqrt(2.0))

# streams 1..4: which get the sqrt2*oh(n0) term via DVE tensor_tensor add,
# vs an extra PE accumulation pass
TT_STREAMS = (1, 2)
PE_STREAMS = (3, 4)

_cache = {}


def _build_nc():
    import concourse.bacc as bacc
    import concourse.tile as tile
    import concourse.mybir as mybir

    nc = bacc.Bacc("TRN2", target_bir_lowering=False, debug=False,
                   num_devices=N_CORES)
    idx_d = nc.dram_tensor("idx", [5, NCORE], mybir.dt.bfloat16,
                           kind="ExternalInput")
    e1_d = nc.dram_tensor("e1", [128, 128], mybir.dt.bfloat16,
                          kind="ExternalInput")
    a_d = nc.dram_tensor("a", [128, 21], mybir.dt.bfloat16,
                         kind="ExternalInput")
    c2_d = nc.dram_tensor("c2", [4, 128, 21], mybir.dt.bfloat16,
                          kind="ExternalInput")
    iota_d = nc.dram_tensor("iota", [128, 1], mybir.dt.float32,
                            kind="ExternalInput")
    out_d = nc.dram_tensor("out", [21, NCORE], mybir.dt.float32,
                           kind="ExternalOutput")
    IE, AD, MU = (mybir.AluOpType.is_equal, mybir.AluOpType.add,
                  mybir.AluOpType.mult)
    RELU = mybir.ActivationFunctionType.Relu

    with tile.TileContext(nc) as tc:
        with (
            tc.tile_pool(name="const", bufs=1) as cpool,
            tc.tile_pool(name="rep", bufs=2) as rpool,
            tc.tile_pool(name="ohp", bufs=2) as opool,
            tc.tile_pool(name="hs", bufs=3) as hpool,
            tc.tile_pool(name="osp", bufs=3) as ospool,
            tc.tile_pool(name="psh", bufs=3, space="PSUM") as phpool,
            tc.tile_pool(name="pso", bufs=2, space="PSUM") as popool,
        ):
            e1_t = cpool.tile([128, 128], mybir.dt.bfloat16)
            nc.sync.dma_start(out=e1_t[:], in_=e1_d.ap())
            a_t = cpool.tile([128, 21], mybir.dt.bfloat16)
            nc.sync.dma_start(out=a_t[:], in_=a_d.ap())
            c2_t = cpool.tile([128, 4 * 21], mybir.dt.bfloat16)
            for s in range(4):
                nc.sync.dma_start(out=c2_t[:, s * 21:(s + 1) * 21],
                                  in_=c2_d.ap()[s])
            iota_t = cpool.tile([128, 1], mybir.dt.float32)
            nc.sync.dma_start(out=iota_t[:], in_=iota_d.ap())

            for sc in range(NCORE // SC):
                ssl = slice(sc * SC, (sc + 1) * SC)
                rep = rpool.tile([128, 5, SC], mybir.dt.bfloat16, tag="rep")
                for s in range(5):
                    nc.sync.dma_start(
                        out=rep[:, s, :],
                        in_=idx_d.ap()[s:s + 1, ssl].broadcast_to([128, SC]))
                # A2 = sqrt2 * oh(n0)
                a2 = opool.tile([128, SC], mybir.dt.bfloat16, tag="a2")
                nc.vector.tensor_scalar(out=a2[:], in0=rep[:, 0, :],
                                        scalar1=iota_t[:], scalar2=SQ2,
                                        op0=IE, op1=MU)
                bs = {}
                for s in range(1, 5):
                    o = opool.tile([128, SC], mybir.dt.bfloat16, tag=f"b{s}")
                    nc.vector.tensor_scalar(out=o[:], in0=rep[:, s, :],
                                            scalar1=iota_t[:], scalar2=None,
                                            op0=IE)
                    if s in TT_STREAMS:
                        b = opool.tile([128, SC], mybir.dt.bfloat16,
                                       tag=f"bb{s}")
                        nc.vector.tensor_tensor(out=b[:], in0=o[:], in1=a2[:],
                                                op=AD)
                        bs[s] = b
                    else:
                        bs[s] = o

                for c in range(SC // CH):
                    csl = slice(c * CH, (c + 1) * CH)
                    gbase = sc * SC + c * CH
                    osl = slice(gbase, gbase + CH)
                    # stage 1: two 2-bank psum tiles, streams (1,2) and (3,4)
                    hsbs = {}
                    for half, pair in enumerate(((1, 2), (3, 4))):
                        ph = phpool.tile([128, 2 * CH], mybir.dt.float32,
                                         tag="ph")
                        for j, s in enumerate(pair):
                            psl = slice(j * CH, (j + 1) * CH)
                            nc.tensor.matmul(out=ph[:, psl], lhsT=e1_t[:],
                                             rhs=bs[s][:, csl],
                                             start=True,
                                             stop=(s not in PE_STREAMS))
                            if s in PE_STREAMS:
                                nc.tensor.matmul(out=ph[:, psl], lhsT=e1_t[:],
                                                 rhs=a2[:, csl],
                                                 start=False, stop=True)
                        hsb = hpool.tile([128, 2 * CH], mybir.dt.bfloat16,
                                         tag=f"hsb{half}")
                        nc.scalar.activation(out=hsb[:], in_=ph[:], func=RELU)
                        hsbs[pair[0]] = hsb[:, 0:CH]
                        hsbs[pair[1]] = hsb[:, CH:2 * CH]
                    # stage 2 + A-term accumulate into po
                    po = popool.tile([21, CH], mybir.dt.float32, tag="po")
                    nc.tensor.matmul(out=po[:], lhsT=a_t[:], rhs=a2[:, csl],
                                     start=True, stop=False)
                    for s in range(1, 5):
                        nc.tensor.matmul(out=po[:],
                                         lhsT=c2_t[:, (s - 1) * 21:s * 21],
                                         rhs=hsbs[s],
                                         start=False, stop=(s == 4))
                    osb = ospool.tile([21, CH], mybir.dt.float32, tag="osb")
                    if c % 2 == 0:
                        nc.vector.tensor_copy(out=osb[:], in_=po[:])
                    else:
                        nc.scalar.copy(out=osb[:], in_=po[:])
                    nc.sync.dma_start(out=out_d.ap()[:, osl], in_=osb[:])
    nc.compile()
    return nc


def _prepare_consts(class_nodes, emb, W1, b1, W2, b2):
    inv_sqrt2 = np.float32(1.0 / np.sqrt(2.0))
    M = np.zeros((5, 5), dtype=np.float32)
    M[0, 0] = 1.0
    for k in range(1, 5):
        M[k, k] = 0.5
        M[k, 0] = inv_sqrt2

    def gcn(x):
        h = np.einsum('ts,...sd->...td', M, x @ W1) + b1
        h = np.maximum(h, 0)
        return np.einsum('ts,...sd->...td', M, h @ W2) + b2

    out_class = gcn(emb[class_nodes]).reshape(21, 105)
    OC = out_class.reshape(21, 5, 21)
    D = np.zeros((21, 5, 21), dtype=np.float32)
    D[:, 0, :] = OC[:, 0, :] + inv_sqrt2 * OC[:, 1:, :].sum(axis=1)
    D[:, 1:, :] = 0.5 * OC[:, 1:, :]
    C2 = np.einsum('kd,ctd->ctk', W2, D)            # [21,5,128]
    K0 = np.einsum('ctd,d->c', OC, b2)              # [21] (zero here)
    E1 = emb @ W1                                   # [129,128]
    # A-term: relu(E1[n0]+b1) @ C2_0 + K0; consumed via A2 = sqrt2*oh(n0),
    # so divide by sqrt2. K0 folded in (K0=0 when b2=0, but keep general).
    A = (np.maximum(E1 + b1, 0) @ C2[:, 0, :].T + K0[None, :])  # [129,21]
    assert abs(A[128]).max() == 0 or b1.any() or b2.any()
    bf = ml_dtypes.bfloat16
    # stage-1 computes 2*h_s; the 0.5 is folded into C2_s here
    c2q = np.ascontiguousarray(np.transpose(0.5 * C2[:, 1:, :], (1, 2, 0)))
    return {
        "e1": E1[:128].astype(bf),
        "a": (A[:128] / np.float32(np.sqrt(2.0))).astype(bf),
        "c2": c2q.astype(bf),                        # [4,128,21]
        "iota": np.arange(128, dtype=np.float32)[:, None],
    }


def kernel(instance_nodes, class_nodes, emb, W1, b1, W2, b2):
    instance_nodes = np.asarray(instance_nodes)
    class_nodes = np.asarray(class_nodes).astype(np.int64)
    emb = np.asarray(emb, dtype=np.float32)
    W1 = np.asarray(W1, dtype=np.float32)
    b1 = np.asarray(b1, dtype=np.float32)
    W2 = np.asarray(W2, dtype=np.float32)
    b2 = np.asarray(b2, dtype=np.float32)

    consts = _prepare_consts(class_nodes, emb, W1, b1, W2, b2)

    # idx rows [5, NTOT] in bf16 (values 0..128 are exact in bf16)
    n = instance_nodes.reshape(NTOT, 5).astype(np.int32)
    idx_bf = np.ascontiguousarray(n.T).astype(ml_dtypes.bfloat16)

    if "nc" not in _cache:
        _cache["nc"] = _build_nc()
    nc = _cache["nc"]

    in_maps = []
    for i in range(N_CORES):
        m = dict(consts)
        m["idx"] = np.ascontiguousarray(
            idx_bf[:, i * NCORE:(i + 1) * NCORE])
        in_maps.append(m)

    from concourse.bass_utils import run_bass_kernel_spmd
    res = run_bass_kernel_spmd(nc, in_maps, list(range(N_CORES)))
    outs = [res.results[i]["out"] for i in range(N_CORES)]   # [21, NCORE] each
    out = np.concatenate(outs, axis=1)                       # [21, NTOT]
    sim = np.ascontiguousarray(out.T).reshape(B, H, W_DIM, 21)
    return sim.astype(np.float32)



# revision 2
# speedup vs baseline: 1.0819x; 1.0819x over previous
"""Trainium2 Bass kernel for nn_Matcher (gnn_message_passing) — v2, fp8 DoubleRow.

Math per graph (indices n0..n4 in [0,129)):
  sim[g,c] = oh(n0) @ A  +  sum_{s=1..4} relu(E1[n_s] + sqrt2*E1[n0]) @ C2_s
with E1 = emb @ W1 (row 128 zero), A and C2_s host-precomputed tables.

v2 device pipeline (per core, data-parallel over graphs):
  - idx rows DMA-replicated across 128 partitions as uint8 (half the bytes)
  - ONE DVE tensor_scalar is_equal per superchunk builds all 5 one-hot
    planes in fp8 (4x perf mode)
  - stage 1: four fp8 DoubleRow matmuls, moving pairs (oh_s, oh_0) against
    stationary [E1; sqrt2*E1] -> 2h_s in one column per graph per stream
  - relu evac: ACT (3 streams) + DVE (1 stream) PSUM->SBUF fp8
  - stage 2: two fp8 DoubleRow matmuls (stream pairs) + one bf16xfp8 A-term
    matmul accumulate into po [21, CH]
  - po evac on DVE as bf16, DMA out, host upcasts to fp32
"""
import numpy as np
import ml_dtypes

N_CORES = 8
B, H, W_DIM = 4, 256, 256
NTOT = B * H * W_DIM            # 262144 graphs
NCORE = NTOT // N_CORES         # 32768
SC = 2048                       # DVE super-chunk (graphs)
CH = 512                        # matmul chunk (graphs)
SQ2 = float(np.sqrt(2.0))

_cache = {}


def _build_nc():
    import concourse.bacc as bacc
    import concourse.tile as tile
    import concourse.mybir as mybir

    nc = bacc.Bacc("TRN2", target_bir_lowering=False, debug=False,
                   num_devices=N_CORES)
    idx_d = nc.dram_tensor("idx", [5, NCORE], mybir.dt.uint8,
                           kind="ExternalInput")
    idxb_d = nc.dram_tensor("idxb", [1, NCORE], mybir.dt.bfloat16,
                            kind="ExternalInput")
    # stage-1 DoubleRow stationary: [128 v, 2, 128 d] = (E1[v,:], sqrt2*E1[v,:])
    e1dr_d = nc.dram_tensor("e1dr", [128, 2, 128], mybir.dt.float8e4,
                            kind="ExternalInput")
    # stage-2 DoubleRow stationaries: pair A=(C2_1,C2_2), pair B=(C2_3,C2_4)
    # padded to 32 output cols (DR stationary free-step must be %16==0)
    c2dr_d = nc.dram_tensor("c2dr", [2, 128, 2, 32], mybir.dt.float8e4,
                            kind="ExternalInput")
    # A-term as split-precision DR: slots (fp8(A), fp8(A - fp8(A))),
    # consumed with a stride-0 moving pair (oh0, oh0)
    adr_d = nc.dram_tensor("adr", [128, 2, 32], mybir.dt.float8e4,
                           kind="ExternalInput")
    iota_d = nc.dram_tensor("iota", [128, 1], mybir.dt.float32,
                            kind="ExternalInput")
    out_d = nc.dram_tensor("out", [21, NCORE], mybir.dt.bfloat16,
                           kind="ExternalOutput")
    IE, MU = mybir.AluOpType.is_equal, mybir.AluOpType.mult
    RELU = mybir.ActivationFunctionType.Relu
    DR = mybir.MatmulPerfMode.DoubleRow
    FP8 = mybir.dt.float8e4

    with tile.TileContext(nc) as tc:
        with (
            tc.tile_pool(name="const", bufs=1) as cpool,
            tc.tile_pool(name="rep", bufs=2) as rpool,
            tc.tile_pool(name="ohp", bufs=2) as opool,
            tc.tile_pool(name="hs", bufs=2) as hpool,
            tc.tile_pool(name="osp", bufs=3) as ospool,
            tc.tile_pool(name="psA", bufs=1, space="PSUM") as pApool,
            tc.tile_pool(name="psB", bufs=1, space="PSUM") as pBpool,
            tc.tile_pool(name="pso", bufs=2, space="PSUM") as popool,
        ):
            e1dr_t = cpool.tile([128, 2, 128], FP8)
            nc.sync.dma_start(out=e1dr_t[:], in_=e1dr_d.ap())
            c2a_t = cpool.tile([128, 2, 32], FP8)
            nc.sync.dma_start(out=c2a_t[:], in_=c2dr_d.ap()[0])
            c2b_t = cpool.tile([128, 2, 32], FP8)
            nc.sync.dma_start(out=c2b_t[:], in_=c2dr_d.ap()[1])
            adr_t = cpool.tile([128, 2, 32], FP8)
            nc.sync.dma_start(out=adr_t[:], in_=adr_d.ap())
            iota_t = cpool.tile([128, 1], mybir.dt.float32)
            nc.sync.dma_start(out=iota_t[:], in_=iota_d.ap())

            # PSUM: phA (streams 1,2) + phB (streams 3,4), 2 banks each
            phA = pApool.tile([128, 2, CH], mybir.dt.float32)
            phB = pBpool.tile([128, 2, CH], mybir.dt.float32)

            # software pipeline: stage2(prev chunk) runs while stage1(cur)
            # fills PSUM; h double-buffered in SBUF
            prev = None  # (h_tile, oh_tile, csl, osl) of previous chunk

            def stage2_and_out(state):
                h, p_oh, p_csl, p_osl = state
                po = popool.tile([32, CH], mybir.dt.float32, tag="po")
                nc.tensor.matmul(out=po[:], lhsT=adr_t[:],
                                 rhs=p_oh[:, 4:5, p_csl].broadcast_to(
                                     [128, 2, CH]),
                                 start=True, stop=False, perf_mode=DR)
                nc.tensor.matmul(out=po[:], lhsT=c2a_t[:],
                                 rhs=h[:, 0:2, :],
                                 start=False, stop=False, perf_mode=DR)
                nc.tensor.matmul(out=po[:], lhsT=c2b_t[:],
                                 rhs=h[:, 2:4, :],
                                 start=False, stop=True, perf_mode=DR)
                osb = ospool.tile([21, CH], mybir.dt.bfloat16, tag="osb")
                nc.vector.tensor_copy(out=osb[:], in_=po[0:21, :])
                nc.sync.dma_start(out=out_d.ap()[:, p_osl], in_=osb[:])

            for sc in range(NCORE // SC):
                ssl = slice(sc * SC, (sc + 1) * SC)
                # slot order: [oh1, oh2, oh3, oh4, oh0, oh0/16] so that
                # t[:, s-1 : 5 : 5-s, :] selects (oh_s, oh_0) and
                # t[:, 4:6, :] selects (oh_0, oh_0/16) for the A DR pair
                rep = rpool.tile([128, 4, SC], mybir.dt.uint8, tag="rep")
                for s in range(1, 5):
                    nc.sync.dma_start(
                        out=rep[:, s - 1, :],
                        in_=idx_d.ap()[s:s + 1, ssl].broadcast_to([128, SC]))
                repb = rpool.tile([128, SC], mybir.dt.bfloat16, tag="repb")
                nc.sync.dma_start(
                    out=repb[:],
                    in_=idxb_d.ap()[0:1, ssl].broadcast_to([128, SC]))
                # one-hot builds on DVE: 4 planes from u8, oh0 from bf16
                oh = opool.tile([128, 5, SC], FP8, tag="oh")
                nc.vector.tensor_scalar(out=oh[:, 0:4, :], in0=rep[:],
                                        scalar1=iota_t[:], scalar2=None,
                                        op0=IE)
                nc.vector.tensor_scalar(out=oh[:, 4, :], in0=repb[:],
                                        scalar1=iota_t[:], scalar2=None,
                                        op0=IE)

                for c in range(SC // CH):
                    csl = slice(c * CH, (c + 1) * CH)
                    gbase = sc * SC + c * CH
                    osl = slice(gbase, gbase + CH)
                    # stage 1: fp8 DoubleRow, pairs (oh_s, oh_0)
                    for ph, streams in ((phA, (1, 2)), (phB, (3, 4))):
                        for k, s in enumerate(streams):
                            nc.tensor.matmul(
                                out=ph[:, k, :],
                                lhsT=e1dr_t[:],
                                rhs=oh[:, s - 1:5:5 - s, csl],
                                start=True, stop=True, perf_mode=DR)
                    if prev is not None:
                        stage2_and_out(prev)
                    h = hpool.tile([128, 4, CH], FP8, tag="h")
                    # relu evac: all on ACT (DVE is build+po bound)
                    nc.scalar.activation(out=h[:, 0:2, :], in_=phA[:],
                                         func=RELU)
                    nc.scalar.activation(out=h[:, 2:4, :], in_=phB[:],
                                         func=RELU)
                    prev = (h, oh, csl, osl)
            stage2_and_out(prev)
    nc.compile()
    return nc


def _prepare_consts(class_nodes, emb, W1, b1, W2, b2):
    inv_sqrt2 = np.float32(1.0 / np.sqrt(2.0))
    M = np.zeros((5, 5), dtype=np.float32)
    M[0, 0] = 1.0
    for k in range(1, 5):
        M[k, k] = 0.5
        M[k, 0] = inv_sqrt2

    def gcn(x):
        h = np.einsum('ts,...sd->...td', M, x @ W1) + b1
        h = np.maximum(h, 0)
        return np.einsum('ts,...sd->...td', M, h @ W2) + b2

    out_class = gcn(emb[class_nodes]).reshape(21, 105)
    OC = out_class.reshape(21, 5, 21)
    D = np.zeros((21, 5, 21), dtype=np.float32)
    D[:, 0, :] = OC[:, 0, :] + inv_sqrt2 * OC[:, 1:, :].sum(axis=1)
    D[:, 1:, :] = 0.5 * OC[:, 1:, :]
    C2 = np.einsum('kd,ctd->ctk', W2, D)            # [21,5,128]
    K0 = np.einsum('ctd,d->c', OC, b2)              # [21] (zero here)
    E1 = emb @ W1                                   # [129,128]
    # A-term: relu(E1[n0]+b1) @ C2_0 + K0, consumed via plain oh(n0)
    A = (np.maximum(E1 + b1, 0) @ C2[:, 0, :].T + K0[None, :])  # [129,21]
    bf = ml_dtypes.bfloat16
    f8 = ml_dtypes.float8_e4m3
    # stage-1 computes 2*h_s = E1[n_s] + sqrt2*E1[n0]; fold the extra 0.5
    # into the stage-2 C2 tables: contributions use 0.5*C2_s vs 2h.
    c2q = 0.5 * C2[:, 1:, :]                        # [21, 4, 128]
    e1dr = np.stack([E1[:128], SQ2 * E1[:128]], axis=1)  # [128, 2, 128]
    c2dr = np.zeros((2, 128, 2, 21), dtype=np.float32)
    for p in range(2):
        for k in range(2):
            c2dr[p, :, k, :] = c2q[:, 2 * p + k, :].T
    c2dr_p = np.zeros((2, 128, 2, 32), dtype=np.float32)
    c2dr_p[:, :, :, :21] = c2dr
    A128 = A[:128]
    A_hi = A128.astype(f8).astype(np.float32)
    adr = np.zeros((128, 2, 32), dtype=np.float32)
    adr[:, 0, :21] = A128
    adr[:, 1, :21] = A128 - A_hi
    return {
        "e1dr": e1dr.astype(f8),
        "c2dr": c2dr_p.astype(f8),
        "adr": adr.astype(f8),
        "iota": np.arange(128, dtype=np.float32)[:, None],
    }


def _prepare_in_maps(inputs):
    """Build the per-core input maps from the full (unsharded) inputs."""
    instance_nodes = np.asarray(inputs["instance_nodes"])
    class_nodes = np.asarray(inputs["class_nodes"]).astype(np.int64)
    emb = np.asarray(inputs["emb"], dtype=np.float32)
    W1 = np.asarray(inputs["W1"], dtype=np.float32)
    b1 = np.asarray(inputs["b1"], dtype=np.float32)
    W2 = np.asarray(inputs["W2"], dtype=np.float32)
    b2 = np.asarray(inputs["b2"], dtype=np.float32)

    consts = _prepare_consts(class_nodes, emb, W1, b1, W2, b2)

    # idx rows [5, NTOT] as uint8 (values 0..128)
    n = instance_nodes.reshape(NTOT, 5).astype(np.int32)
    idx_u8 = np.ascontiguousarray(n.T).astype(np.uint8)

    idx_bf = idx_u8[0:1].astype(ml_dtypes.bfloat16)
    in_maps = []
    for i in range(N_CORES):
        m = dict(consts)
        m["idx"] = np.ascontiguousarray(
            idx_u8[:, i * NCORE:(i + 1) * NCORE])
        m["idxb"] = np.ascontiguousarray(
            idx_bf[:, i * NCORE:(i + 1) * NCORE])
        in_maps.append(m)
    return in_maps


def kernel(instance_nodes, class_nodes, emb, W1, b1, W2, b2):
    in_maps = _prepare_in_maps({
        "instance_nodes": instance_nodes, "class_nodes": class_nodes,
        "emb": emb, "W1": W1, "b1": b1, "W2": W2, "b2": b2})

    if "nc" not in _cache:
        _cache["nc"] = _build_nc()
    nc = _cache["nc"]

    from concourse.bass_utils import run_bass_kernel_spmd
    res = run_bass_kernel_spmd(nc, in_maps, list(range(N_CORES)))
    outs = [res.results[i]["out"] for i in range(N_CORES)]   # [21, NCORE] each
    out = np.concatenate(outs, axis=1).astype(np.float32)    # [21, NTOT]
    sim = np.ascontiguousarray(out.T).reshape(B, H, W_DIM, 21)
    return sim


# revision 4
# speedup vs baseline: 1.1279x; 1.0425x over previous
"""Trainium2 Bass kernel for nn_Matcher (gnn_message_passing) — fp8 DoubleRow.

Math per graph (indices n0..n4 in [0,129)):
  sim[g,c] = oh(n0) @ A  +  sum_{s=1..4} relu(E1[n_s] + sqrt2*E1[n0]) @ C2_s
with E1 = emb @ W1 (row 128 is zero), A and C2_s host-precomputed tables
folding the class branch, GCN propagation weights, W2 and all scaling.

Device pipeline (per core, data-parallel over graphs; 7 PE columns/graph
instead of the naive 11, ~half the HBM traffic):
  - idx rows DMA-replicated across 128 partitions as uint8
  - one fused DVE tensor_scalar is_equal per superchunk builds 4 one-hot
    planes in fp8 (2x perf mode); oh(n0) built from a bf16 replica
  - stage 1: four fp8 DoubleRow matmuls; moving pairs (oh_s, oh_0) against
    the 256-deep stationary [E1; sqrt2*E1] compute 2h_s in one column each
  - relu evacuation: ACT activations PSUM->SBUF fp8
  - stage 2: three fp8 DoubleRow matmuls into po[32, CH]: the A-term uses a
    stride-0 moving pair (oh0, oh0) against the split-precision stationary
    (fp8(A), fp8(A - fp8(A))); C2 stream pairs contract the h pairs
  - po evacuated on DVE as bf16; host upcasts to fp32
Software-pipelined one chunk deep (stage2 consumes the previous chunk's h
while stage1 fills PSUM). Measured ~183us vs the 243us bf16 baseline.
"""
import numpy as np
import ml_dtypes

N_CORES = 8
B, H, W_DIM = 4, 256, 256
NTOT = B * H * W_DIM            # 262144 graphs
NCORE = NTOT // N_CORES         # 32768
SC = 2048                       # DVE super-chunk (graphs)
CH = 512                        # matmul chunk (graphs)
SQ2 = float(np.sqrt(2.0))

_cache = {}


def _build_nc():
    import concourse.bacc as bacc
    import concourse.tile as tile
    import concourse.mybir as mybir

    nc = bacc.Bacc("TRN2", target_bir_lowering=False, debug=False,
                   num_devices=N_CORES)
    idx_d = nc.dram_tensor("idx", [5, NCORE], mybir.dt.uint8,
                           kind="ExternalInput")
    idxb_d = nc.dram_tensor("idxb", [1, NCORE], mybir.dt.bfloat16,
                            kind="ExternalInput")
    # stage-1 DoubleRow stationary: [128 v, 2, 128 d] = (E1[v,:], sqrt2*E1[v,:])
    e1dr_d = nc.dram_tensor("e1dr", [128, 2, 128], mybir.dt.float8e4,
                            kind="ExternalInput")
    # stage-2 DoubleRow stationaries: pair A=(C2_1,C2_2), pair B=(C2_3,C2_4)
    # padded to 32 output cols (DR stationary free-step must be %16==0)
    c2dr_d = nc.dram_tensor("c2dr", [2, 128, 2, 32], mybir.dt.float8e4,
                            kind="ExternalInput")
    # A-term as split-precision DR: slots (fp8(A), fp8(A - fp8(A))),
    # consumed with a stride-0 moving pair (oh0, oh0)
    adr_d = nc.dram_tensor("adr", [128, 2, 32], mybir.dt.float8e4,
                           kind="ExternalInput")
    iota_d = nc.dram_tensor("iota", [128, 1], mybir.dt.float32,
                            kind="ExternalInput")
    out_d = nc.dram_tensor("out", [21, NCORE], mybir.dt.bfloat16,
                           kind="ExternalOutput")
    IE, MU = mybir.AluOpType.is_equal, mybir.AluOpType.mult
    RELU = mybir.ActivationFunctionType.Relu
    DR = mybir.MatmulPerfMode.DoubleRow
    FP8 = mybir.dt.float8e4

    with tile.TileContext(nc) as tc:
        with (
            tc.tile_pool(name="const", bufs=1) as cpool,
            tc.tile_pool(name="rep", bufs=2) as rpool,
            tc.tile_pool(name="ohp", bufs=2) as opool,
            tc.tile_pool(name="hs", bufs=2) as hpool,
            tc.tile_pool(name="osp", bufs=3) as ospool,
            tc.tile_pool(name="psA", bufs=1, space="PSUM") as pApool,
            tc.tile_pool(name="psB", bufs=1, space="PSUM") as pBpool,
            tc.tile_pool(name="pso", bufs=2, space="PSUM") as popool,
        ):
            e1dr_t = cpool.tile([128, 2, 128], FP8)
            nc.sync.dma_start(out=e1dr_t[:], in_=e1dr_d.ap())
            c2a_t = cpool.tile([128, 2, 32], FP8)
            nc.sync.dma_start(out=c2a_t[:], in_=c2dr_d.ap()[0])
            c2b_t = cpool.tile([128, 2, 32], FP8)
            nc.sync.dma_start(out=c2b_t[:], in_=c2dr_d.ap()[1])
            adr_t = cpool.tile([128, 2, 32], FP8)
            nc.sync.dma_start(out=adr_t[:], in_=adr_d.ap())
            iota_t = cpool.tile([128, 1], mybir.dt.float32)
            nc.sync.dma_start(out=iota_t[:], in_=iota_d.ap())

            # PSUM: phA (streams 1,2) + phB (streams 3,4), 2 banks each
            phA = pApool.tile([128, 2, CH], mybir.dt.float32)
            phB = pBpool.tile([128, 2, CH], mybir.dt.float32)

            # software pipeline: stage2(prev chunk) runs while stage1(cur)
            # fills PSUM; h double-buffered in SBUF
            prev = None  # (h_tile, oh_tile, csl, osl) of previous chunk

            def stage2_and_out(state):
                h, p_oh, p_csl, p_osl = state
                po = popool.tile([32, CH], mybir.dt.float32, tag="po")
                nc.tensor.matmul(out=po[:], lhsT=adr_t[:],
                                 rhs=p_oh[:, 4:5, p_csl].broadcast_to(
                                     [128, 2, CH]),
                                 start=True, stop=False, perf_mode=DR)
                nc.tensor.matmul(out=po[:], lhsT=c2a_t[:],
                                 rhs=h[:, 0:2, :],
                                 start=False, stop=False, perf_mode=DR)
                nc.tensor.matmul(out=po[:], lhsT=c2b_t[:],
                                 rhs=h[:, 2:4, :],
                                 start=False, stop=True, perf_mode=DR)
                osb = ospool.tile([21, CH], mybir.dt.bfloat16, tag="osb")
                nc.vector.tensor_copy(out=osb[:], in_=po[0:21, :])
                nc.sync.dma_start(out=out_d.ap()[:, p_osl], in_=osb[:])

            def issue_sc_load(sc):
                # slot order: [oh1, oh2, oh3, oh4, oh0];
                # t[:, s-1 : 5 : 5-s, :] selects (oh_s, oh_0)
                ssl = slice(sc * SC, (sc + 1) * SC)
                rep = rpool.tile([128, 4, SC], mybir.dt.uint8,
                                 name="rep", tag="rep")
                for s in range(1, 5):
                    nc.sync.dma_start(
                        out=rep[:, s - 1, :],
                        in_=idx_d.ap()[s:s + 1, ssl].broadcast_to([128, SC]))
                repb = rpool.tile([128, SC], mybir.dt.bfloat16,
                                  name="repb", tag="repb")
                nc.sync.dma_start(
                    out=repb[:],
                    in_=idxb_d.ap()[0:1, ssl].broadcast_to([128, SC]))
                # one-hot builds on DVE: 4 planes from u8, oh0 from bf16
                oh = opool.tile([128, 5, SC], FP8, name="oh", tag="oh")
                nc.vector.tensor_scalar(out=oh[:, 0:4, :], in0=rep[:],
                                        scalar1=iota_t[:], scalar2=None,
                                        op0=IE)
                nc.vector.tensor_scalar(out=oh[:, 4, :], in0=repb[:],
                                        scalar1=iota_t[:], scalar2=None,
                                        op0=IE)
                return oh

            n_sc = NCORE // SC
            oh_next = issue_sc_load(0)
            for sc in range(n_sc):
                oh = oh_next

                for c in range(SC // CH):
                    # prefetch next superchunk's build mid-SC so the DVE
                    # finishes it before chunk 0 of sc+1 needs it
                    if c == 1 and sc + 1 < n_sc:
                        oh_next = issue_sc_load(sc + 1)
                    csl = slice(c * CH, (c + 1) * CH)
                    gbase = sc * SC + c * CH
                    osl = slice(gbase, gbase + CH)
                    # stage 1: fp8 DoubleRow, pairs (oh_s, oh_0)
                    for ph, streams in ((phA, (1, 2)), (phB, (3, 4))):
                        for k, s in enumerate(streams):
                            nc.tensor.matmul(
                                out=ph[:, k, :],
                                lhsT=e1dr_t[:],
                                rhs=oh[:, s - 1:5:5 - s, csl],
                                start=True, stop=True, perf_mode=DR)
                    if prev is not None:
                        stage2_and_out(prev)
                    h = hpool.tile([128, 4, CH], FP8, tag="h")
                    # relu evac: all on ACT (DVE is build+po bound)
                    nc.scalar.activation(out=h[:, 0:2, :], in_=phA[:],
                                         func=RELU)
                    nc.scalar.activation(out=h[:, 2:4, :], in_=phB[:],
                                         func=RELU)
                    prev = (h, oh, csl, osl)
            stage2_and_out(prev)
    nc.compile()
    return nc


def _prepare_consts(class_nodes, emb, W1, b1, W2, b2):
    inv_sqrt2 = np.float32(1.0 / np.sqrt(2.0))
    M = np.zeros((5, 5), dtype=np.float32)
    M[0, 0] = 1.0
    for k in range(1, 5):
        M[k, k] = 0.5
        M[k, 0] = inv_sqrt2

    def gcn(x):
        h = np.einsum('ts,...sd->...td', M, x @ W1) + b1
        h = np.maximum(h, 0)
        return np.einsum('ts,...sd->...td', M, h @ W2) + b2

    out_class = gcn(emb[class_nodes]).reshape(21, 105)
    OC = out_class.reshape(21, 5, 21)
    D = np.zeros((21, 5, 21), dtype=np.float32)
    D[:, 0, :] = OC[:, 0, :] + inv_sqrt2 * OC[:, 1:, :].sum(axis=1)
    D[:, 1:, :] = 0.5 * OC[:, 1:, :]
    C2 = np.einsum('kd,ctd->ctk', W2, D)            # [21,5,128]
    K0 = np.einsum('ctd,d->c', OC, b2)              # [21] (zero here)
    E1 = emb @ W1                                   # [129,128]
    # A-term: relu(E1[n0]+b1) @ C2_0 + K0, consumed via plain oh(n0)
    A = (np.maximum(E1 + b1, 0) @ C2[:, 0, :].T + K0[None, :])  # [129,21]
    bf = ml_dtypes.bfloat16
    f8 = ml_dtypes.float8_e4m3
    # stage-1 computes 2*h_s = E1[n_s] + sqrt2*E1[n0]; fold the extra 0.5
    # into the stage-2 C2 tables: contributions use 0.5*C2_s vs 2h.
    c2q = 0.5 * C2[:, 1:, :]                        # [21, 4, 128]
    e1dr = np.stack([E1[:128], SQ2 * E1[:128]], axis=1)  # [128, 2, 128]
    c2dr = np.zeros((2, 128, 2, 21), dtype=np.float32)
    for p in range(2):
        for k in range(2):
            c2dr[p, :, k, :] = c2q[:, 2 * p + k, :].T
    c2dr_p = np.zeros((2, 128, 2, 32), dtype=np.float32)
    c2dr_p[:, :, :, :21] = c2dr
    A128 = A[:128]
    A_hi = A128.astype(f8).astype(np.float32)
    adr = np.zeros((128, 2, 32), dtype=np.float32)
    adr[:, 0, :21] = A128
    adr[:, 1, :21] = A128 - A_hi
    return {
        "e1dr": e1dr.astype(f8),
        "c2dr": c2dr_p.astype(f8),
        "adr": adr.astype(f8),
        "iota": np.arange(128, dtype=np.float32)[:, None],
    }


def _prepare_in_maps(inputs):
    """Build the per-core input maps from the full (unsharded) inputs."""
    instance_nodes = np.asarray(inputs["instance_nodes"])
    class_nodes = np.asarray(inputs["class_nodes"]).astype(np.int64)
    emb = np.asarray(inputs["emb"], dtype=np.float32)
    W1 = np.asarray(inputs["W1"], dtype=np.float32)
    b1 = np.asarray(inputs["b1"], dtype=np.float32)
    W2 = np.asarray(inputs["W2"], dtype=np.float32)
    b2 = np.asarray(inputs["b2"], dtype=np.float32)

    consts = _prepare_consts(class_nodes, emb, W1, b1, W2, b2)

    # idx rows [5, NTOT] as uint8 (values 0..128)
    n = instance_nodes.reshape(NTOT, 5).astype(np.int32)
    idx_u8 = np.ascontiguousarray(n.T).astype(np.uint8)

    idx_bf = idx_u8[0:1].astype(ml_dtypes.bfloat16)
    in_maps = []
    for i in range(N_CORES):
        m = dict(consts)
        m["idx"] = np.ascontiguousarray(
            idx_u8[:, i * NCORE:(i + 1) * NCORE])
        m["idxb"] = np.ascontiguousarray(
            idx_bf[:, i * NCORE:(i + 1) * NCORE])
        in_maps.append(m)
    return in_maps


def kernel(instance_nodes, class_nodes, emb, W1, b1, W2, b2):
    in_maps = _prepare_in_maps({
        "instance_nodes": instance_nodes, "class_nodes": class_nodes,
        "emb": emb, "W1": W1, "b1": b1, "W2": W2, "b2": b2})

    if "nc" not in _cache:
        _cache["nc"] = _build_nc()
    nc = _cache["nc"]

    from concourse.bass_utils import run_bass_kernel_spmd
    res = run_bass_kernel_spmd(nc, in_maps, list(range(N_CORES)))
    outs = [res.results[i]["out"] for i in range(N_CORES)]   # [21, NCORE] each
    out = np.concatenate(outs, axis=1).astype(np.float32)    # [21, NTOT]
    sim = np.ascontiguousarray(out.T).reshape(B, H, W_DIM, 21)
    return sim
